# revision 7
# baseline (speedup 1.0000x reference)
"""Trainium2 Bass kernel for nn_AdvancedAttention (llama-style GQA attention
block, B=2, S=2048, D=4096, 32 Q heads / 8 KV heads, head_dim 128, RoPE,
additive mask, fresh cache start_pos=0).

Sharding: tensor-parallel over heads across 8 NeuronCores. Core c owns
Q heads 4c..4c+3 and KV head c (wq/wk/wv output dim sharded), plus the
matching 512-column slice of wo's input dim. Each core computes a partial
[D, B*S] output; the host sums the 8 partials and transposes back.

Matmuls run in float32r (single-pass fp32 on the PE, ~1.5e-4 rel err) with
fp32 PSUM accumulation. The additive mask is applied as a post-exp multiply
by exp(mask). When the mask verifies as causal-block-compatible on the host,
fully-masked score tiles are skipped and the multiply runs only on diagonal
tiles; otherwise a no-skip variant applies exp(mask) on every tile.
"""
import math
import numpy as np

import concourse.tile as tile
from concourse import bacc, mybir
from concourse.bass_utils import run_bass_kernel_spmd
from concourse.masks import make_identity

F32 = mybir.dt.float32
F32R = mybir.dt.float32r

B, S, D = 2, 2048, 4096
T = B * S                      # 4096 tokens (batch-major)
HD = 128                       # head dim
NQH, NKVH = 32, 8              # global head counts
QH = NQH // 8                  # 4 q heads per core
OC = QH * HD + 2 * HD          # 768 projection channels per core (v,k,4q)
NKT = D // 128                 # 32 contraction tiles for projections
TB = 512                       # token block (phase 1)
NTB = T // TB                  # 8
TQB = 512                      # query block (phase 2)
NTQB = S // TQB                # 4 per batch
NSKT = S // 128                # 16 key tiles per batch
NDIAG = TQB // 128             # 4 diagonal key tiles per query block

_CACHE = {}


def _ntk(tqb, causal):
    """Number of key tiles computed for query block tqb."""
    return min(NSKT, (tqb + 1) * TQB // 128) if causal else NSKT


def _build_nc(causal):
    nc = bacc.Bacc("TRN2", target_bir_lowering=False, debug=False)

    xT = nc.dram_tensor("xT", [D, T], F32R, kind="ExternalInput").ap()
    wqkvT = nc.dram_tensor("wqkvT", [D, OC], F32R, kind="ExternalInput").ap()
    woT = nc.dram_tensor("woT", [QH * HD, D], F32R, kind="ExternalInput").ap()
    # exp(mask^T): causal variant ships only the diagonal 512x512 blocks
    em_cols = TQB if causal else S
    emT = nc.dram_tensor("emT", [S, em_cols], F32R, kind="ExternalInput").ap()
    cs = nc.dram_tensor("cs", [128, S], F32, kind="ExternalInput").ap()
    sn = nc.dram_tensor("sn", [128, S], F32, kind="ExternalInput").ap()
    outT = nc.dram_tensor("outT", [D, T], F32, kind="ExternalOutput").ap()

    xR = xT.rearrange("(nk p) t -> p nk t", p=128)
    wR = wqkvT.rearrange("(nk p) o -> p nk o", p=128)
    woR = woT.rearrange("(a p) d -> p a d", p=128)
    emR = emT.rearrange("(a p) q -> p a q", p=128)

    with tile.TileContext(nc) as tc:
        with tc.tile_pool(name="dram", bufs=1, space="DRAM") as dpool, \
             tc.tile_pool(name="resid", bufs=1) as rpool0:
            # Q^T spill (roped), [4 heads x 128, T]
            spill = dpool.tile([QH * HD, T], F32R)
            spR = spill.rearrange("(a p) t -> p a t", p=128)

            kt_sb = [rpool0.tile([128, S], F32R, name=f"ktb{b}")
                     for b in range(B)]
            vT_sb = [rpool0.tile([128, S], F32R, name=f"vtb{b}")
                     for b in range(B)]
            v_sb = [rpool0.tile([128, NSKT, 128], F32R, name=f"vb{b}")
                    for b in range(B)]
            ident = rpool0.tile([128, 128], F32R)
            ones = rpool0.tile([128, 1], F32R)
            # first query block's exp-mask tiles (hoisted for early DMA)
            tqb_first = NTQB - 1
            nd0 = NDIAG if causal else NSKT
            em0 = rpool0.tile([128, nd0, TQB], F32R, name="em0")

            # ------------- phase 1: QKV projection + rope -------------
            with tc.tile_pool(name="p1", bufs=1) as pool1, \
                 tc.tile_pool(name="p1x", bufs=8) as xpool, \
                 tc.tile_pool(name="p1s", bufs=3) as spool, \
                 tc.tile_pool(name="p1o", bufs=4) as opool, \
                 tc.tile_pool(name="p1t", bufs=2) as tpool, \
                 tc.tile_pool(name="ps1", bufs=1, space="PSUM") as ppool1:

                wt = pool1.tile([128, NKT, OC], F32R)
                for kt in range(NKT):
                    eng = nc.scalar if kt % 2 == 0 else nc.gpsimd
                    eng.dma_start(out=wt[:, kt, :], in_=wR[:, kt, :])
                if causal:
                    nc.gpsimd.dma_start(
                        out=em0,
                        in_=emR[:, NDIAG * tqb_first:NDIAG * (tqb_first + 1), :])
                else:
                    nc.gpsimd.dma_start(
                        out=em0, in_=emR[:, :, tqb_first * TQB:(tqb_first + 1) * TQB])
                cs_sb = pool1.tile([128, S], F32)
                nc.gpsimd.dma_start(out=cs_sb, in_=cs)
                sn_sb = pool1.tile([128, S], F32)
                nc.gpsimd.dma_start(out=sn_sb, in_=sn)
                ident_f = pool1.tile([128, 128], F32)
                make_identity(nc, ident_f)
                nc.vector.tensor_copy(ident, ident_f)
                ones_f = pool1.tile([128, 1], F32)
                nc.vector.memset(ones_f, 1.0)
                nc.vector.tensor_copy(ones, ones_f)

                def rope(dst, src, csl):
                    tmp = tpool.tile([128, TB], F32R, name="tmp")
                    nc.vector.tensor_mul(dst[0:64, :], src[0:64, :],
                                         cs_sb[0:64, csl])
                    nc.vector.tensor_mul(tmp[0:64, :], src[64:128, :],
                                         sn_sb[64:128, csl])
                    nc.vector.tensor_sub(dst[0:64, :], dst[0:64, :],
                                         tmp[0:64, :])
                    nc.vector.tensor_mul(dst[64:128, :], src[0:64, :],
                                         sn_sb[0:64, csl])
                    nc.vector.tensor_mul(tmp[64:128, :], src[64:128, :],
                                         cs_sb[64:128, csl])
                    nc.vector.tensor_add(dst[64:128, :], dst[64:128, :],
                                         tmp[64:128, :])

                for tb in range(NTB):
                    b = tb // NTQB
                    s_off = (tb % NTQB) * TB
                    tsl = slice(tb * TB, (tb + 1) * TB)
                    ssl = slice(s_off, s_off + TB)
                    # column order in wqkvT: [v, k, q0..q3]
                    pss = [ppool1.tile([128, TB], F32, name=f"psq{tb}_{i}",
                                       tag=f"psq{i}",
                                       bufs=(2 if i in (2, 5) else 1))
                           for i in range(6)]
                    for kt in range(NKT):
                        xt = xpool.tile([128, TB], F32R, name="xt")
                        nc.sync.dma_start(out=xt, in_=xR[:, kt, tsl])
                        for ot in range(6):
                            nc.tensor.matmul(
                                pss[ot], wt[:, kt, ot * 128:(ot + 1) * 128],
                                xt, start=(kt == 0), stop=(kt == NKT - 1))
                    # v: plain copy into resident vT, then transpose this
                    # token block's 4 tiles into V-natural layout
                    nc.scalar.copy(vT_sb[b][:, ssl], pss[0])
                    for j in range(TB // 128):
                        tkt = s_off // 128 + j
                        ps_t = ppool1.tile([128, 128], F32R,
                                           name=f"ps_t{tb}_{j}",
                                           tag="psq5", bufs=2)
                        nc.tensor.transpose(
                            ps_t,
                            vT_sb[b][:, s_off + j * 128:s_off + (j + 1) * 128],
                            ident)
                        nc.scalar.copy(v_sb[b][:, tkt, :], ps_t)
                    # k, q: ACT-evict PSUM to SBUF stage, then rope on DVE
                    stage_k = spool.tile([128, TB], F32, name="stage")
                    nc.scalar.copy(stage_k, pss[1])
                    rope(kt_sb[b][:, ssl], stage_k, ssl)
                    for h in range(QH):
                        stage = spool.tile([128, TB], F32, name="stage")
                        nc.scalar.copy(stage, pss[2 + h])
                        ro = opool.tile([128, TB], F32R, name="ro")
                        rope(ro, stage, ssl)
                        nc.gpsimd.dma_start(out=spR[:, h, tsl], in_=ro)

            # ------------- phase 2: attention + wo -------------
            with tc.tile_pool(name="p2", bufs=1) as pool2, \
                 tc.tile_pool(name="p2m", bufs=2) as mpool, \
                 tc.tile_pool(name="p2q", bufs=3) as qpool, \
                 tc.tile_pool(name="p2p", bufs=4) as ppool, \
                 tc.tile_pool(name="p2a", bufs=2) as apool, \
                 tc.tile_pool(name="p2r", bufs=2) as rpool, \
                 tc.tile_pool(name="p2f", bufs=4) as fpool, \
                 tc.tile_pool(name="ps2", bufs=1, space="PSUM") as ppool2:

                wo_sb = pool2.tile([128, QH, D], F32R)
                nc.scalar.dma_start(out=wo_sb, in_=woR)

                def emit_wo(g, lo, hi):
                    tqb, b, attn = g
                    for dt_ in range(lo, hi):
                        ps_f = ppool2.tile([128, TQB], F32, tag="ps_s", bufs=3,
                                           name=f"psf{tqb}{b}{dt_}")
                        for ot in range(QH):
                            nc.tensor.matmul(
                                ps_f, wo_sb[:, ot, dt_ * 128:(dt_ + 1) * 128],
                                attn[ot], start=(ot == 0), stop=(ot == QH - 1))
                        fst = fpool.tile([128, TQB], F32, name="fst")
                        if dt_ % 2 == 0:
                            nc.scalar.copy(fst, ps_f)
                        else:
                            nc.vector.tensor_copy(fst, ps_f)
                        nc.sync.dma_start(
                            out=outT[dt_ * 128:(dt_ + 1) * 128,
                                     b * S + tqb * TQB:b * S + (tqb + 1) * TQB],
                            in_=fst)

                prev = None
                em = em0
                for tqb in [3, 2, 1, 0]:
                    ntk = _ntk(tqb, causal)
                    diag_lo = ntk - NDIAG if causal else 0  # first masked tile
                    if tqb != tqb_first:
                        nd = NDIAG if causal else NSKT
                        em = mpool.tile([128, nd, TQB], F32R, name="em")
                        if causal:
                            nc.gpsimd.dma_start(
                                out=em,
                                in_=emR[:, NDIAG * tqb:NDIAG * (tqb + 1), :])
                        else:
                            nc.gpsimd.dma_start(
                                out=em,
                                in_=emR[:, :, tqb * TQB:(tqb + 1) * TQB])
                    for b in range(B):
                        attn = []
                        for h in range(QH):
                            qt = qpool.tile([128, TQB], F32R, name="qt")
                            nc.sync.dma_start(
                                out=qt,
                                in_=spR[:, h, b * S + tqb * TQB:
                                        b * S + (tqb + 1) * TQB])
                            ps_o = ppool2.tile([128, TQB], F32, tag="ps_o",
                                               bufs=3, name=f"pso{tqb}{b}{h}")
                            ps_sum = ppool2.tile([1, TQB], F32, tag="ps_sum",
                                                 bufs=2, name=f"psm{tqb}{b}{h}")
                            pes = [None] * ntk

                            def av(t):
                                nc.tensor.matmul(ps_o, v_sb[b][:, t, :],
                                                 pes[t], start=(t == 0),
                                                 stop=(t == ntk - 1))
                                nc.tensor.matmul(ps_sum, ones, pes[t],
                                                 start=(t == 0),
                                                 stop=(t == ntk - 1))

                            for tkt in range(ntk):
                                ps_s = ppool2.tile([128, TQB], F32, tag="ps_s",
                                                   bufs=3,
                                                   name=f"pss{tqb}{b}{h}{tkt}")
                                nc.tensor.matmul(
                                    ps_s,
                                    kt_sb[b][:, tkt * 128:(tkt + 1) * 128],
                                    qt, start=True, stop=True)
                                pe = ppool.tile([128, TQB], F32R, name="pe")
                                nc.scalar.activation(
                                    pe, ps_s, mybir.ActivationFunctionType.Exp)
                                if tkt >= diag_lo:
                                    nc.vector.tensor_mul(
                                        pe, pe, em[:, tkt - diag_lo, :])
                                pes[tkt] = pe
                                if tkt >= 1:
                                    av(tkt - 1)
                            av(ntk - 1)
                            rec = rpool.tile([1, TQB], F32, name="rec")
                            nc.vector.reciprocal_approx_fast(rec, ps_sum)
                            rec128 = rpool.tile([128, TQB], F32, name="rec128")
                            nc.gpsimd.partition_broadcast(rec128, rec)
                            at = apool.tile([128, TQB], F32R, name=f"attn{h}",
                                            tag=f"attn{h}")
                            nc.vector.tensor_mul(at, ps_o, rec128)
                            attn.append(at)
                            if prev is not None:
                                emit_wo(prev, 8 * h, 8 * (h + 1))
                        prev = (tqb, b, attn)
                emit_wo(prev, 0, D // 128)
    nc.compile()
    return nc


def _get_nc(causal):
    key = f"nc_{causal}"
    if key not in _CACHE:
        _CACHE[key] = _build_nc(causal)
    return _CACHE[key]


def _mask_is_causal_compatible(mask):
    """True if skipped tiles are fully masked AND off-diagonal computed
    tiles carry exactly zero mask (so the multiply can be elided)."""
    for tqb in range(NTQB):
        ntk = _ntk(tqb, True)
        qs = slice(tqb * TQB, (tqb + 1) * TQB)
        if ntk * 128 < S and mask[qs, ntk * 128:].max() > -1e30:
            return False
        lo = (ntk - NDIAG) * 128
        if lo > 0 and np.any(mask[qs, :lo] != 0.0):
            return False
    return True


def make_in_maps(x, mask, freqs_cos, freqs_sin, wq, wk, wv, wo, causal):
    """Host-side sharding/layout prep. Returns list of 8 per-core maps."""
    perm = np.concatenate([np.arange(0, HD, 2), np.arange(1, HD, 2)])
    xT = np.ascontiguousarray(np.asarray(x, np.float32).reshape(T, D).T)
    mask = np.asarray(mask, np.float32)
    if causal:
        # diagonal 512x512 blocks of mask^T, stacked: emT[tqb*512+tk, tq]
        emT = np.concatenate(
            [np.exp(np.minimum(
                mask[t * TQB:(t + 1) * TQB, t * TQB:(t + 1) * TQB].T, 80.0))
             for t in range(NTQB)], 0).astype(np.float32)
    else:
        emT = np.exp(np.minimum(mask.T, 80.0)).astype(np.float32)
    emT = np.ascontiguousarray(emT)
    cosT = np.asarray(freqs_cos, np.float32).T          # [64, S]
    sinT = np.asarray(freqs_sin, np.float32).T
    cs = np.concatenate([cosT, cosT], 0)                # [128, S]
    sn = np.concatenate([sinT, sinT], 0)
    wq_r = np.asarray(wq, np.float32).reshape(NQH, HD, D) / math.sqrt(HD)
    wk_r = np.asarray(wk, np.float32).reshape(NKVH, HD, D)
    wv_r = np.asarray(wv, np.float32).reshape(NKVH, HD, D)
    wo_a = np.asarray(wo, np.float32)

    in_maps = []
    for c in range(8):
        wq_c = wq_r[c * QH:(c + 1) * QH][:, perm, :].reshape(QH * HD, D)
        wk_c = wk_r[c][perm, :]
        wv_c = wv_r[c]
        wqkvT = np.ascontiguousarray(
            np.concatenate([wv_c, wk_c, wq_c], 0).T)    # [D, OC] v,k,q order
        woT = np.ascontiguousarray(wo_a[:, c * QH * HD:(c + 1) * QH * HD].T)
        in_maps.append({
            "xT": xT, "wqkvT": wqkvT, "woT": woT,
            "emT": emT, "cs": cs, "sn": sn,
        })
    return in_maps


def combine_outputs(results):
    """Sum per-core partial outputs and restore [B, S, D] layout."""
    total = results[0]["outT"].astype(np.float64)
    for c in range(1, 8):
        total += results[c]["outT"]
    return np.ascontiguousarray(total.T).reshape(B, S, D).astype(np.float32)


def kernel(x, mask, freqs_cos, freqs_sin, wq, wk, wv, wo, start_pos):
    assert int(start_pos) == 0, "kernel compiled for start_pos == 0"
    causal = _mask_is_causal_compatible(np.asarray(mask, np.float32))
    nc = _get_nc(causal)
    in_maps = make_in_maps(x, mask, freqs_cos, freqs_sin, wq, wk, wv, wo,
                           causal)
    res = run_bass_kernel_spmd(nc, in_maps, core_ids=list(range(8)))
    return combine_outputs(res.results)


# revision 8
# speedup vs baseline: 1.1842x; 1.1842x over previous
"""Trainium2 Bass kernel for nn_AdvancedAttention (llama-style GQA attention
block, B=2, S=2048, D=4096, 32 Q heads / 8 KV heads, head_dim 128, RoPE,
additive mask, fresh cache start_pos=0).

Sharding: tensor-parallel over heads across 8 NeuronCores. Core c owns
Q heads 4c..4c+3 and KV head c (wq/wk/wv output dim sharded), plus the
matching 512-column slice of wo's input dim. Each core computes a partial
[D, B*S] output; the host sums the 8 partials and transposes back.

Matmuls run in float32r (single-pass fp32 on the PE, ~1.5e-4 rel err) with
fp32 PSUM accumulation. The additive mask is applied as a post-exp multiply
by exp(mask). When the mask verifies as causal-block-compatible on the host,
fully-masked score tiles are skipped and the multiply runs only on diagonal
tiles; otherwise a no-skip variant applies exp(mask) on every tile.
"""
import math
import numpy as np

import concourse.tile as tile
from concourse import bacc, mybir
from concourse.bass_utils import run_bass_kernel_spmd
from concourse.masks import make_identity

F32 = mybir.dt.float32
F32R = mybir.dt.float32r

B, S, D = 2, 2048, 4096
T = B * S                      # 4096 tokens (batch-major)
HD = 128                       # head dim
NQH, NKVH = 32, 8              # global head counts
QH = NQH // 8                  # 4 q heads per core
OC = QH * HD + 2 * HD          # 768 projection channels per core (v,k,4q)
NKT = D // 128                 # 32 contraction tiles for projections
TB = 512                       # token block (phase 1)
NTB = T // TB                  # 8
TQB = 512                      # query block (phase 2)
NTQB = S // TQB                # 4 per batch
NSKT = S // 128                # 16 key tiles per batch
NDIAG = TQB // 128             # 4 diagonal key tiles per query block

_CACHE = {}


def _ntk(tqb, causal):
    """Number of key tiles computed for query block tqb."""
    return min(NSKT, (tqb + 1) * TQB // 128) if causal else NSKT


def _build_nc(causal):
    nc = bacc.Bacc("TRN2", target_bir_lowering=False, debug=False)

    xT = nc.dram_tensor("xT", [D, T], F32R, kind="ExternalInput").ap()
    wqkvT = nc.dram_tensor("wqkvT", [D, OC], F32R, kind="ExternalInput").ap()
    woT = nc.dram_tensor("woT", [QH * HD, D], F32R, kind="ExternalInput").ap()
    # exp(mask^T): causal variant ships only the diagonal 512x512 blocks
    em_cols = TQB if causal else S
    emT = nc.dram_tensor("emT", [S, em_cols], F32R, kind="ExternalInput").ap()
    cs = nc.dram_tensor("cs", [128, S], F32, kind="ExternalInput").ap()
    sn = nc.dram_tensor("sn", [128, S], F32, kind="ExternalInput").ap()
    outT = nc.dram_tensor("outT", [D, T], F32, kind="ExternalOutput").ap()

    xR = xT.rearrange("(nk p) t -> p nk t", p=128)
    wR = wqkvT.rearrange("(nk p) o -> p nk o", p=128)
    woR = woT.rearrange("(a p) d -> p a d", p=128)
    emR = emT.rearrange("(a p) q -> p a q", p=128)

    with tile.TileContext(nc) as tc:
        with tc.tile_pool(name="dram", bufs=1, space="DRAM") as dpool, \
             tc.tile_pool(name="resid", bufs=1) as rpool0:
            # Q^T spill (roped), [4 heads x 128, T]
            spill = dpool.tile([QH * HD, T], F32R)
            spR = spill.rearrange("(a p) t -> p a t", p=128)

            kt_sb = [rpool0.tile([128, S], F32R, name=f"ktb{b}")
                     for b in range(B)]
            vT_sb = [rpool0.tile([128, S], F32R, name=f"vtb{b}")
                     for b in range(B)]
            v_sb = [rpool0.tile([128, NSKT, 128], F32R, name=f"vb{b}")
                    for b in range(B)]
            ident = rpool0.tile([128, 128], F32R)
            ones = rpool0.tile([128, 1], F32R)
            # first query block's exp-mask tiles (hoisted for early DMA)
            tqb_first = NTQB - 1
            nd0 = NDIAG if causal else NSKT
            em0 = rpool0.tile([128, nd0, TQB], F32R, name="em0")

            # ------------- phase 1: QKV projection + rope -------------
            with tc.tile_pool(name="p1", bufs=1) as pool1, \
                 tc.tile_pool(name="p1x", bufs=8) as xpool, \
                 tc.tile_pool(name="p1s", bufs=3) as spool, \
                 tc.tile_pool(name="p1o", bufs=4) as opool, \
                 tc.tile_pool(name="p1t", bufs=2) as tpool, \
                 tc.tile_pool(name="ps1", bufs=1, space="PSUM") as ppool1:

                wt = pool1.tile([128, NKT, OC], F32R)
                for kt in range(NKT):
                    eng = nc.scalar if kt % 2 == 0 else nc.gpsimd
                    eng.dma_start(out=wt[:, kt, :], in_=wR[:, kt, :])
                if causal:
                    nc.gpsimd.dma_start(
                        out=em0,
                        in_=emR[:, NDIAG * tqb_first:NDIAG * (tqb_first + 1), :])
                else:
                    nc.gpsimd.dma_start(
                        out=em0, in_=emR[:, :, tqb_first * TQB:(tqb_first + 1) * TQB])
                cs_sb = pool1.tile([128, S], F32)
                nc.gpsimd.dma_start(out=cs_sb, in_=cs)
                sn_sb = pool1.tile([128, S], F32)
                nc.gpsimd.dma_start(out=sn_sb, in_=sn)
                ident_f = pool1.tile([128, 128], F32)
                make_identity(nc, ident_f)
                nc.vector.tensor_copy(ident, ident_f)
                ones_f = pool1.tile([128, 1], F32)
                nc.vector.memset(ones_f, 1.0)
                nc.vector.tensor_copy(ones, ones_f)

                def rope(dst, src, csl):
                    tmp = tpool.tile([128, TB], F32R, name="tmp")
                    nc.vector.tensor_mul(dst[0:64, :], src[0:64, :],
                                         cs_sb[0:64, csl])
                    nc.vector.tensor_mul(tmp[0:64, :], src[64:128, :],
                                         sn_sb[64:128, csl])
                    nc.vector.tensor_sub(dst[0:64, :], dst[0:64, :],
                                         tmp[0:64, :])
                    nc.vector.tensor_mul(dst[64:128, :], src[0:64, :],
                                         sn_sb[0:64, csl])
                    nc.vector.tensor_mul(tmp[64:128, :], src[64:128, :],
                                         cs_sb[64:128, csl])
                    nc.vector.tensor_add(dst[64:128, :], dst[64:128, :],
                                         tmp[64:128, :])

                for tb in range(NTB):
                    b = tb // NTQB
                    s_off = (tb % NTQB) * TB
                    tsl = slice(tb * TB, (tb + 1) * TB)
                    ssl = slice(s_off, s_off + TB)
                    # column order in wqkvT: [v, k, q0..q3]
                    pss = [ppool1.tile([128, TB], F32, name=f"psq{tb}_{i}",
                                       tag=f"psq{i}",
                                       bufs=(2 if i in (2, 5) else 1))
                           for i in range(6)]
                    for kt in range(NKT):
                        xt = xpool.tile([128, TB], F32R, name="xt")
                        nc.sync.dma_start(out=xt, in_=xR[:, kt, tsl])
                        for ot in range(6):
                            nc.tensor.matmul(
                                pss[ot], wt[:, kt, ot * 128:(ot + 1) * 128],
                                xt, start=(kt == 0), stop=(kt == NKT - 1))
                    # v: plain copy into resident vT
                    nc.scalar.copy(vT_sb[b][:, ssl], pss[0])
                    # after last tb of each batch: transpose V^T -> V tiles,
                    # 4 transposes per PSUM bank, single ACT eviction
                    if tb % NTQB == NTQB - 1:
                        for grp in range(NSKT // 4):
                            ps_t4 = ppool1.tile([128, 4, 128], F32R,
                                                name=f"ps_t{b}_{grp}",
                                                tag="psq5", bufs=2)
                            for j in range(4):
                                tkt = grp * 4 + j
                                nc.tensor.transpose(
                                    ps_t4[:, j, :],
                                    vT_sb[b][:, tkt * 128:(tkt + 1) * 128],
                                    ident)
                            nc.scalar.copy(
                                v_sb[b][:, grp * 4:(grp + 1) * 4, :], ps_t4)
                    # k, q: ACT-evict PSUM to SBUF stage, then rope on DVE
                    stage_k = spool.tile([128, TB], F32, name="stage")
                    nc.scalar.copy(stage_k, pss[1])
                    rope(kt_sb[b][:, ssl], stage_k, ssl)
                    for h in range(QH):
                        stage = spool.tile([128, TB], F32, name="stage")
                        nc.scalar.copy(stage, pss[2 + h])
                        ro = opool.tile([128, TB], F32R, name="ro")
                        rope(ro, stage, ssl)
                        nc.gpsimd.dma_start(out=spR[:, h, tsl], in_=ro)

            # ------------- phase 2: attention + wo -------------
            with tc.tile_pool(name="p2", bufs=1) as pool2, \
                 tc.tile_pool(name="p2m", bufs=2) as mpool, \
                 tc.tile_pool(name="p2q", bufs=3) as qpool, \
                 tc.tile_pool(name="p2p", bufs=4) as ppool, \
                 tc.tile_pool(name="p2a", bufs=2) as apool, \
                 tc.tile_pool(name="p2r", bufs=2) as rpool, \
                 tc.tile_pool(name="p2f", bufs=4) as fpool, \
                 tc.tile_pool(name="ps2", bufs=1, space="PSUM") as ppool2:

                wo_sb = pool2.tile([128, QH, D], F32R)
                nc.scalar.dma_start(out=wo_sb, in_=woR)

                def emit_wo(g, lo, hi):
                    tqb, b, attn = g
                    for dt_ in range(lo, hi):
                        ps_f = ppool2.tile([128, TQB], F32, tag="ps_s", bufs=3,
                                           name=f"psf{tqb}{b}{dt_}")
                        for ot in range(QH):
                            nc.tensor.matmul(
                                ps_f, wo_sb[:, ot, dt_ * 128:(dt_ + 1) * 128],
                                attn[ot], start=(ot == 0), stop=(ot == QH - 1))
                        fst = fpool.tile([128, TQB], F32, name="fst")
                        if dt_ % 2 == 0:
                            nc.scalar.copy(fst, ps_f)
                        else:
                            nc.vector.tensor_copy(fst, ps_f)
                        nc.sync.dma_start(
                            out=outT[dt_ * 128:(dt_ + 1) * 128,
                                     b * S + tqb * TQB:b * S + (tqb + 1) * TQB],
                            in_=fst)

                prev = None
                em = em0
                for tqb in [3, 2, 1, 0]:
                    ntk = _ntk(tqb, causal)
                    diag_lo = ntk - NDIAG if causal else 0  # first masked tile
                    if tqb != tqb_first:
                        nd = NDIAG if causal else NSKT
                        em = mpool.tile([128, nd, TQB], F32R, name="em")
                        if causal:
                            nc.gpsimd.dma_start(
                                out=em,
                                in_=emR[:, NDIAG * tqb:NDIAG * (tqb + 1), :])
                        else:
                            nc.gpsimd.dma_start(
                                out=em,
                                in_=emR[:, :, tqb * TQB:(tqb + 1) * TQB])
                    for b in range(B):
                        attn = []
                        for h in range(QH):
                            qt = qpool.tile([128, TQB], F32R, name="qt")
                            nc.sync.dma_start(
                                out=qt,
                                in_=spR[:, h, b * S + tqb * TQB:
                                        b * S + (tqb + 1) * TQB])
                            ps_o = ppool2.tile([128, TQB], F32, tag="ps_o",
                                               bufs=3, name=f"pso{tqb}{b}{h}")
                            ps_sum = ppool2.tile([1, TQB], F32, tag="ps_sum",
                                                 bufs=2, name=f"psm{tqb}{b}{h}")
                            pes = [None] * ntk

                            def av(t):
                                nc.tensor.matmul(ps_o, v_sb[b][:, t, :],
                                                 pes[t], start=(t == 0),
                                                 stop=(t == ntk - 1))
                                nc.tensor.matmul(ps_sum, ones, pes[t],
                                                 start=(t == 0),
                                                 stop=(t == ntk - 1))

                            for tkt in range(ntk):
                                ps_s = ppool2.tile([128, TQB], F32, tag="ps_s",
                                                   bufs=3,
                                                   name=f"pss{tqb}{b}{h}{tkt}")
                                nc.tensor.matmul(
                                    ps_s,
                                    kt_sb[b][:, tkt * 128:(tkt + 1) * 128],
                                    qt, start=True, stop=True)
                                pe = ppool.tile([128, TQB], F32R, name="pe")
                                nc.scalar.activation(
                                    pe, ps_s, mybir.ActivationFunctionType.Exp)
                                if tkt >= diag_lo:
                                    nc.vector.tensor_mul(
                                        pe, pe, em[:, tkt - diag_lo, :])
                                pes[tkt] = pe
                                if tkt >= 1:
                                    av(tkt - 1)
                            av(ntk - 1)
                            rec = rpool.tile([1, TQB], F32, name="rec")
                            nc.vector.reciprocal_approx_fast(rec, ps_sum)
                            rec128 = rpool.tile([128, TQB], F32, name="rec128")
                            nc.gpsimd.partition_broadcast(rec128, rec)
                            at = apool.tile([128, TQB], F32R, name=f"attn{h}",
                                            tag=f"attn{h}")
                            nc.vector.tensor_mul(at, ps_o, rec128)
                            attn.append(at)
                            if prev is not None:
                                emit_wo(prev, 8 * h, 8 * (h + 1))
                        prev = (tqb, b, attn)
                emit_wo(prev, 0, D // 128)
    nc.compile()
    return nc


def _get_nc(causal):
    key = f"nc_{causal}"
    if key not in _CACHE:
        _CACHE[key] = _build_nc(causal)
    return _CACHE[key]


def _mask_is_causal_compatible(mask):
    """True if skipped tiles are fully masked AND off-diagonal computed
    tiles carry exactly zero mask (so the multiply can be elided)."""
    for tqb in range(NTQB):
        ntk = _ntk(tqb, True)
        qs = slice(tqb * TQB, (tqb + 1) * TQB)
        if ntk * 128 < S and mask[qs, ntk * 128:].max() > -1e30:
            return False
        lo = (ntk - NDIAG) * 128
        if lo > 0 and np.any(mask[qs, :lo] != 0.0):
            return False
    return True


def make_in_maps(x, mask, freqs_cos, freqs_sin, wq, wk, wv, wo, causal):
    """Host-side sharding/layout prep. Returns list of 8 per-core maps."""
    perm = np.concatenate([np.arange(0, HD, 2), np.arange(1, HD, 2)])
    xT = np.ascontiguousarray(np.asarray(x, np.float32).reshape(T, D).T)
    mask = np.asarray(mask, np.float32)
    if causal:
        # diagonal 512x512 blocks of mask^T, stacked: emT[tqb*512+tk, tq]
        emT = np.concatenate(
            [np.exp(np.minimum(
                mask[t * TQB:(t + 1) * TQB, t * TQB:(t + 1) * TQB].T, 80.0))
             for t in range(NTQB)], 0).astype(np.float32)
    else:
        emT = np.exp(np.minimum(mask.T, 80.0)).astype(np.float32)
    emT = np.ascontiguousarray(emT)
    cosT = np.asarray(freqs_cos, np.float32).T          # [64, S]
    sinT = np.asarray(freqs_sin, np.float32).T
    cs = np.concatenate([cosT, cosT], 0)                # [128, S]
    sn = np.concatenate([sinT, sinT], 0)
    wq_r = np.asarray(wq, np.float32).reshape(NQH, HD, D) / math.sqrt(HD)
    wk_r = np.asarray(wk, np.float32).reshape(NKVH, HD, D)
    wv_r = np.asarray(wv, np.float32).reshape(NKVH, HD, D)
    wo_a = np.asarray(wo, np.float32)

    in_maps = []
    for c in range(8):
        wq_c = wq_r[c * QH:(c + 1) * QH][:, perm, :].reshape(QH * HD, D)
        wk_c = wk_r[c][perm, :]
        wv_c = wv_r[c]
        wqkvT = np.ascontiguousarray(
            np.concatenate([wv_c, wk_c, wq_c], 0).T)    # [D, OC] v,k,q order
        woT = np.ascontiguousarray(wo_a[:, c * QH * HD:(c + 1) * QH * HD].T)
        in_maps.append({
            "xT": xT, "wqkvT": wqkvT, "woT": woT,
            "emT": emT, "cs": cs, "sn": sn,
        })
    return in_maps


def combine_outputs(results):
    """Sum per-core partial outputs and restore [B, S, D] layout."""
    total = results[0]["outT"].astype(np.float64)
    for c in range(1, 8):
        total += results[c]["outT"]
    return np.ascontiguousarray(total.T).reshape(B, S, D).astype(np.float32)


def kernel(x, mask, freqs_cos, freqs_sin, wq, wk, wv, wo, start_pos):
    assert int(start_pos) == 0, "kernel compiled for start_pos == 0"
    causal = _mask_is_causal_compatible(np.asarray(mask, np.float32))
    nc = _get_nc(causal)
    in_maps = make_in_maps(x, mask, freqs_cos, freqs_sin, wq, wk, wv, wo,
                           causal)
    res = run_bass_kernel_spmd(nc, in_maps, core_ids=list(range(8)))
    return combine_outputs(res.results)
